# revision 37
# baseline (speedup 1.0000x reference)
"""CQAttention (BiDAF context-query attention) Bass kernel for Trainium2.

Contract: kernel(C, Q, W) takes the FULL inputs
    C [64, 256, 1024] f32, Q [64, 256, 128] f32, W [64, 1, 768] f32
and returns the FULL output [64, 1024, 1024] f32 matching reference.py.

Strategy: data-parallel over the batch dim across 8 NeuronCores
(8 batches per core). Per batch, everything is fused on-chip:
  S^T = (wm*Q)^T @ C + sq + sc   (float32r matmuls, fp32 PSUM)
  row/col softmax without max-subtraction (values are small),
  A = Q @ S_^T, M = C @ S__, B = M @ S_^T,
  out = concat([C, A, C*A, C*B]) * dropout_mask * (1/0.9)
The dropout mask (jax threefry, key 42) is input-independent; it is
computed host-side once on CPU and shipped to the kernel as a u8 tensor.

Softmax normalizers: the column softmax (over context, for S__) uses the
exp instruction's fused accumulator; the row softmax denominator is a
ones-vector matmul of expS^T (0.9-weighted, folding dropout's 1/0.9),
reciprocated via exp(-ln(x)) on ScalarE and broadcast back over the 128
partitions with a K=1 matmul, so S_^T needs no transposes at all.
"""

import functools
import os
import sys

sys.path.insert(0, "/opt/trn_rl_repo")

import numpy as np

import concourse.bass as bass
import concourse.mybir as mybir
import concourse.tile as tile
from concourse import bacc
from concourse.bass_utils import run_bass_kernel_spmd
from concourse.masks import make_identity

# Reorder the activation-function tables so the one set containing
# Copy+Exp+Ln+Identity is preferred for every activation: otherwise the
# table pick alternates between "exp_and_others" and "natural_log" and
# each Exp<->Ln transition costs a ~1.3us table reload on ScalarE.
_orig_get_act_tables = bacc.get_activation_tables


def _act_tables_ln_exp_first(arch):
    # Order must be preserved (act_func_set_id is positional in
    # act_info.json); instead strip our functions from every other set so
    # the selection pass lands on the combined set.
    import concourse.mybir as _mb

    ours = {
        _mb.ActivationFunctionType.Exp,
        _mb.ActivationFunctionType.Ln,
        _mb.ActivationFunctionType.Copy,
        _mb.ActivationFunctionType.Identity,
    }
    tabs = _orig_get_act_tables(arch)
    out = {}
    for name, funcs in tabs.items():
        if name == "natural_log_exp_and_others":
            out[name] = funcs
        else:
            out[name] = funcs - ours
    return out


bacc.get_activation_tables = _act_tables_ln_exp_first

B, D, LC, LQ = 64, 256, 1024, 128
NCORES = 8
BPC = B // NCORES  # batches per core
P = 128
KO = D // P  # 2 d-chunks
CI = LC // P  # 8 context chunks
QO = 4 * D // P  # 8 output row chunks
INV_KEEP = float(1.0 / 0.9)

F32 = mybir.dt.float32
F32R = mybir.dt.float32r
U8 = mybir.dt.uint8
Exp = mybir.ActivationFunctionType.Exp
Ln = mybir.ActivationFunctionType.Ln
Copy = mybir.ActivationFunctionType.Copy
MULT = mybir.AluOpType.mult

LAST_RESULTS = None  # BassKernelResults of the most recent run (for test.py)


def _emit_batch(nc, pools, aps, b):
    """Emit all instructions for one batch b (0..BPC-1)."""
    io, conv, mid, outp, tmpp, ps_big, ps_med, consts = pools
    C_ap, Q_ap, Wt_ap, M_ap, O_ap, ident_r, ones_r, ones09_c, ident_f = aps

    # ---- loads -----------------------------------------------------------
    C_sb = io.tile([P, KO, LC], F32, tag="C_sb", bufs=4)
    C_hbm = C_ap[b].rearrange("(ko p) i -> p ko i", p=P)
    nc.sync.dma_start(C_sb[:, :, 0:512], C_hbm[:, :, 0:512])
    nc.sync.dma_start(C_sb[:, :, 512:LC], C_hbm[:, :, 512:LC])
    Q_sb = io.tile([P, KO, LQ], F32, tag="Q_sb")
    nc.gpsimd.dma_start(Q_sb[:], Q_ap[b].rearrange("(ko p) j -> p ko j", p=P))
    Wt_sb = io.tile([P, 6], F32, tag="Wt_sb")
    nc.gpsimd.dma_start(Wt_sb[:], Wt_ap[b])
    M_sb = io.tile([P, QO, LC], U8, tag="M_sb", bufs=3)
    M_hbm = M_ap[b].rearrange("(qo p) i -> p qo i", p=P)
    nc.scalar.dma_start(M_sb[:, 0:4], M_hbm[:, 0:4])
    nc.scalar.dma_start(M_sb[:, 4:8], M_hbm[:, 4:8])

    # ---- f32r conversions ------------------------------------------------
    C_r = conv.tile([P, KO, LC], F32R, tag="C_r")
    nc.scalar.activation(C_r[:, :, 0:512], C_sb[:, :, 0:512], Copy)
    nc.scalar.activation(C_r[:, :, 512:LC], C_sb[:, :, 512:LC], Copy)
    Wt_r = conv.tile([P, 6], F32R, tag="Wt_r")
    nc.vector.tensor_copy(Wt_r[:], Wt_sb[:])
    # Qw = wm * Q  (wm is per-partition within each d-chunk)
    Qw_r = conv.tile([P, KO, LQ], F32R, tag="Qw_r")
    for ko in range(KO):
        nc.scalar.activation(
            Qw_r[:, ko], Q_sb[:, ko], Copy, scale=Wt_sb[:, 4 + ko : 5 + ko]
        )

    # ---- sq[j] = wq . Q[:, j]  (as a per-partition column) ---------------
    # (f32r matmuls need a moving free size >= 2; col 1 is junk, ignored)
    sq_ps = ps_med.tile([P, 512], F32, tag="med")
    for ko in range(KO):
        nc.tensor.matmul(
            sq_ps[:, 0:2],
            Q_sb[:, ko],
            Wt_sb[:, ko : ko + 2],
            start=(ko == 0),
            stop=(ko == KO - 1),
        )
    sq_sb = mid.tile([P, 1], F32, tag="sq_sb")
    nc.scalar.activation(sq_sb[:], sq_ps[:, 0:1], Copy)

    # ---- sc[i] = wc . C[:, i]  (as a [1, LC] row) ------------------------
    sc_r = mid.tile([1, LC], F32R, tag="sc_r")
    for n in range(2):
        sl = slice(512 * n, 512 * (n + 1))
        sc_ps = ps_med.tile([1, 512], F32, tag="med")
        for ko in range(KO):
            nc.tensor.matmul(
                sc_ps[:],
                Wt_r[:, 2 + ko : 3 + ko],
                C_r[:, ko, sl],
                start=(ko == 0),
                stop=(ko == KO - 1),
            )
        nc.scalar.activation(sc_r[:, sl], sc_ps[:], Copy)

    # ---- S^T = Qw^T @ C + ones^T @ sc  ([LQ part, LC free], PSUM) --------
    smt_ps = ps_big.tile([P, LC], F32, tag="big")
    for n in range(2):
        sl = slice(512 * n, 512 * (n + 1))
        for ko in range(KO):
            nc.tensor.matmul(
                smt_ps[:, sl],
                Qw_r[:, ko],
                C_r[:, ko, sl],
                start=(ko == 0),
                stop=False,
            )
        nc.tensor.matmul(
            smt_ps[:, sl], ones_r[:], sc_r[:, sl], start=False, stop=True
        )

    # ---- expS^T = Exp(S^T + sq), fused colsum -> rcol --------------------
    expst_sb = mid.tile([P, LC], F32R, tag="expst_sb")
    csum2 = mid.tile([P, 2], F32, tag="csum2")
    for n in range(2):
        sl = slice(512 * n, 512 * (n + 1))
        nc.scalar.activation(
            expst_sb[:, sl],
            smt_ps[:, sl],
            Exp,
            bias=sq_sb[:],
            accum_out=csum2[:, n : n + 1],
        )
    csum = mid.tile([P, 1], F32, tag="csum")
    nc.vector.scalar_tensor_tensor(
        csum[:], csum2[:, 0:1], 1.0, csum2[:, 1:2], MULT, mybir.AluOpType.add
    )
    rcol = mid.tile([P, 1], F32, tag="rcol")
    nc.vector.reciprocal(rcol[:], csum[:])

    # ---- row softmax denominator: 0.9 * rowsum, reciprocal, broadcast ----
    # rs_row[0, i] = 0.9 * sum_j expS^T[j, i]   (ones-column matmul)
    rr_row = mid.tile([1, LC], F32R, tag="rr_row")
    for n in range(2):
        sl = slice(512 * n, 512 * (n + 1))
        rs_ps = ps_med.tile([1, 512], F32, tag="med")
        nc.tensor.matmul(rs_ps[:], ones09_c[:], expst_sb[:, sl], start=True, stop=True)
        # 1/x = exp(-ln(x)) on ScalarE (vector.reciprocal would serialize
        # on one partition; ACT has slack). ln stays fp32: rounding it to
        # f32r would be amplified by |ln| ~ 6 in the exp.
        lnr = mid.tile([1, 512], F32, tag="lnr")
        nc.scalar.activation(lnr[:], rs_ps[:], Ln)
        nc.scalar.activation(rr_row[:, sl], lnr[:], Exp, scale=-1.0)

    # ---- S_^T = expS^T * rr_row (broadcast over partitions via K=1 mm) ---
    s1t_sb = mid.tile([P, LC], F32R, tag="s1t_sb")
    for n in range(2):
        sl = slice(512 * n, 512 * (n + 1))
        rrb_ps = ps_med.tile([P, 512], F32, tag="med")
        nc.tensor.matmul(rrb_ps[:], ones_r[:], rr_row[:, sl], start=True, stop=True)
        nc.vector.tensor_tensor(s1t_sb[:, sl], expst_sb[:, sl], rrb_ps[:], MULT)

    # ---- transpose expS^T -> expS (natural) for the Mt contraction -------
    exps_ps = ps_big.tile([P, LC], F32R, tag="big")
    for ci in range(CI):
        nc.tensor.matmul(
            exps_ps[:, P * ci : P * (ci + 1)],
            expst_sb[:, P * ci : P * (ci + 1)],
            ident_r[:],
            is_transpose=True,
            start=True,
            stop=True,
        )
    exps_sb = mid.tile([P, CI, P], mybir.dt.bfloat16, tag="exps_sb")
    for n in range(2):
        nc.scalar.activation(
            exps_sb[:, 4 * n : 4 * (n + 1)],
            exps_ps[:, 512 * n : 512 * (n + 1)].rearrange(
                "p (ci q) -> p ci q", q=P
            ),
            Copy,
        )

    # ---- Qt = Q^T  ([LQ part, D free]) -----------------------------------
    qt_ps = ps_med.tile([P, 256], F32, tag="med")
    for ko in range(KO):
        nc.tensor.matmul(
            qt_ps[:, P * ko : P * (ko + 1)],
            Q_sb[:, ko],
            ident_f[:],
            is_transpose=True,
            start=True,
            stop=True,
        )
    qt_sb = mid.tile([P, D], F32R, tag="qt_sb")
    nc.scalar.activation(qt_sb[:], qt_ps[:], Copy)

    # ---- Ct = C^T  ([LC part chunks, D free]) ----------------------------
    ct_sb = mid.tile([P, CI, D], mybir.dt.bfloat16, tag="ct_sb")
    for ko in range(KO):
        tk_ps = ps_big.tile([P, LC], F32R, tag="big")
        for ci in range(CI):
            nc.tensor.matmul(
                tk_ps[:, P * ci : P * (ci + 1)],
                C_r[:, ko, P * ci : P * (ci + 1)],
                ident_r[:],
                is_transpose=True,
                start=True,
                stop=True,
            )
        nc.scalar.activation(
            ct_sb[:, :, P * ko : P * (ko + 1)],
            tk_ps[:].rearrange("p (ci q) -> p ci q", q=P),
            Copy,
        )

    # ---- Mt[j,d] = rcol[j] * sum_k expS[k,j] Ct[k,d] ---------------------
    mt_ps = ps_med.tile([P, 256], F32, tag="med")
    for ci in range(CI):
        nc.tensor.matmul(
            mt_ps[:, :D],
            exps_sb[:, ci],
            ct_sb[:, ci],
            start=(ci == 0),
            stop=(ci == CI - 1),
        )
    mt_sb = mid.tile([P, D], F32R, tag="mt_sb")
    nc.scalar.activation(mt_sb[:], mt_ps[:, :D], Copy, scale=rcol[:])

    # ---- A, B and the four output quarters -------------------------------
    o_view = O_ap[b].rearrange("(qo p) i -> p qo i", p=P)
    stage0 = outp.tile([P, 4, LC], F32, tag="stage0")
    stage1 = outp.tile([P, 4, LC], F32, tag="stage1")
    for h in range(KO):
        nc.vector.scalar_tensor_tensor(
            stage0[:, h], C_sb[:, h], INV_KEEP, M_sb[:, h], MULT, MULT
        )
    for h in range(KO):
        a_ps = ps_big.tile([P, LC], F32, tag="big")
        b_ps = ps_big.tile([P, LC], F32, tag="big")
        for n in range(2):
            sl = slice(512 * n, 512 * (n + 1))
            nc.tensor.matmul(
                a_ps[:, sl],
                qt_sb[:, P * h : P * (h + 1)],
                s1t_sb[:, sl],
                start=True,
                stop=True,
            )
            nc.tensor.matmul(
                b_ps[:, sl],
                mt_sb[:, P * h : P * (h + 1)],
                s1t_sb[:, sl],
                start=True,
                stop=True,
            )
        # q1 = A * m1   (A already carries the 1/0.9 via rr_row)
        nc.vector.tensor_tensor(stage0[:, 2 + h], a_ps[:], M_sb[:, 2 + h], MULT)
        # q2 = (C * A) * m2
        t2 = tmpp.tile([P, LC], F32, tag="tmp")
        nc.vector.tensor_tensor(t2[:], C_sb[:, h], a_ps[:], MULT)
        nc.gpsimd.tensor_tensor(stage1[:, h], t2[:], M_sb[:, 4 + h], MULT)
        # q3 = (C * B) * m3
        t3 = tmpp.tile([P, LC], F32, tag="tmp")
        nc.vector.tensor_tensor(t3[:], C_sb[:, h], b_ps[:], MULT)
        nc.vector.tensor_tensor(stage1[:, 2 + h], t3[:], M_sb[:, 6 + h], MULT)

    nc.sync.dma_start(o_view[:, 0:4], stage0[:])
    nc.sync.dma_start(o_view[:, 4:8], stage1[:])


@functools.cache
def _build(loop_r: int = 1):
    nc = bacc.Bacc("TRN2", target_bir_lowering=False, debug=False)

    C_ap = nc.dram_tensor("C", [BPC, D, LC], F32, kind="ExternalInput").ap()
    Q_ap = nc.dram_tensor("Q", [BPC, D, LQ], F32, kind="ExternalInput").ap()
    Wt_ap = nc.dram_tensor("Wt", [BPC, P, 6], F32, kind="ExternalInput").ap()
    M_ap = nc.dram_tensor("mask", [BPC, 4 * D, LC], U8, kind="ExternalInput").ap()
    O_ap = nc.dram_tensor("out", [BPC, 4 * D, LC], F32, kind="ExternalOutput").ap()

    with tile.TileContext(nc) as tc:
        with (
            tc.tile_pool(name="consts", bufs=1) as consts,
            tc.tile_pool(name="io", bufs=2) as io,
            tc.tile_pool(name="conv", bufs=2) as conv,
            tc.tile_pool(name="mid", bufs=2) as mid,
            tc.tile_pool(name="outp", bufs=2) as outp,
            tc.tile_pool(name="tmpp", bufs=2) as tmpp,
            tc.tile_pool(name="ps_big", bufs=2, space="PSUM") as ps_big,
            tc.tile_pool(name="ps_med", bufs=4, space="PSUM") as ps_med,
        ):
            ident_f = consts.tile([P, P], F32)
            make_identity(nc, ident_f[:])
            ident_r = consts.tile([P, P], F32R)
            nc.vector.tensor_copy(ident_r[:], ident_f[:])
            ones_f = consts.tile([1, P], F32)
            nc.gpsimd.memset(ones_f[:], 1.0)
            ones_r = consts.tile([1, P], F32R)
            nc.vector.tensor_copy(ones_r[:], ones_f[:])
            # column of 0.9 (weights the row-sum so A, B carry 1/0.9)
            o9_f = consts.tile([P, 1], F32)
            nc.gpsimd.memset(o9_f[:], 0.9)
            ones09_c = consts.tile([P, 1], F32R)
            nc.vector.tensor_copy(ones09_c[:], o9_f[:])

            pools = (io, conv, mid, outp, tmpp, ps_big, ps_med, consts)
            aps = (C_ap, Q_ap, Wt_ap, M_ap, O_ap, ident_r, ones_r, ones09_c, ident_f)
            if loop_r > 1:
                _hints = (
                    mybir.EngineType.PE,
                    mybir.EngineType.DVE,
                    mybir.EngineType.Activation,
                    mybir.EngineType.SP,
                    mybir.EngineType.Pool,
                )
                with tc.For_i(0, loop_r, 1, hint_engines=_hints):
                    for b in range(BPC):
                        _emit_batch(nc, pools, aps, b)
            else:
                for b in range(BPC):
                    _emit_batch(nc, pools, aps, b)

    nc.compile()
    return nc


@functools.cache
def _dropout_mask():
    import jax

    with jax.default_device(jax.devices("cpu")[0]):
        keep = jax.random.bernoulli(jax.random.key(42), 0.9, (B, 4 * D, LC))
        return np.asarray(keep).astype(np.uint8)


def kernel(C, Q, W):
    global LAST_RESULTS
    C = np.ascontiguousarray(C, dtype=np.float32)
    Q = np.ascontiguousarray(Q, dtype=np.float32)
    W = np.ascontiguousarray(W, dtype=np.float32)
    mask = _dropout_mask()
    # W[b, 0, x*128 + p] -> Wt[b, p, x]; columns: wq0 wq1 wc0 wc1 wm0 wm1
    Wt = np.ascontiguousarray(W[:, 0, :].reshape(B, 6, P).transpose(0, 2, 1))

    nc = _build()
    in_maps = []
    for k in range(NCORES):
        s = slice(k * BPC, (k + 1) * BPC)
        in_maps.append(
            {
                "C": C[s],
                "Q": Q[s],
                "Wt": Wt[s],
                "mask": np.ascontiguousarray(mask[s]),
            }
        )
    LAST_RESULTS = run_bass_kernel_spmd(
        nc,
        in_maps,
        list(range(NCORES)),
        trace=bool(os.environ.get("CQA_TRACE")),
    )
    out = np.concatenate([r["out"] for r in LAST_RESULTS.results], axis=0)
    return out
